# revision 1
# baseline (speedup 1.0000x reference)
"""Trainium2 Bass kernel for nn_DecoderGRU (attention GRU decoder + vocab head).

Strategy (8 NeuronCores, data-parallel over batch, 8 rows/core):
  - Feature-major layouts ([feature-on-partition, r/batch-on-free]); the GRU
    gates come out feature-major directly, so no transposes anywhere.
  - Hoisted out of the 32-step time loop:
      * feat_proj = features @ attn_W[:E] + attn_b   (fp32r matmul, once)
      * xgx       = emb @ W_ih[:, :E].T + b_ih+b_hh  (fp32r matmul, once)
      * logits    = h_all @ fc_W + fc_b              (fp16 matmul, at end)
  - Per step: h_proj/gh/cgx as bf16 weight-stationary matmuls (FWL); energy
    add + tanh + scores pipelined in two r-halves; softmax without max-sub
    (scores are O(1)); attention weights scattered across partitions by a
    tiny SBUF->SBUF DMA; context as 32 rank-1 PE matmuls contracting r;
    sigmoid via 0.5*(1+tanh(x/2)) so ACT stays on one table set.
"""

import threading

import numpy as np
import ml_dtypes

B, R, E, H, V, L = 64, 49, 512, 512, 10000, 33
T = L - 1            # 32 decode steps
NCORES = 8
BL = B // NCORES     # 8 batch rows per core
KT = E // 128        # 4 k-tiles of 128 for E=H=512
M3H = (3 * H) // 128  # 12 m-tiles for gate dim
RSPLIT = ((0, 25), (25, 49))  # r-halves for the energy pipeline

_BUILD_LOCK = threading.Lock()
_BUILT = {}


def _round_f32r(a):
    """fp32r rounding (drop 13 low mantissa bits, round-to-nearest) on host.

    The BIR verifier requires fp32r-matmul inputs to be produced already
    rounded; for DMA-fed tensors that producer is the host.
    """
    v = np.ascontiguousarray(a, dtype=np.float32).view(np.uint32).astype(np.uint64)
    v = (v + 0x1000) & 0xFFFFE000
    return v.astype(np.uint32).view(np.float32)


def _build(has_fcb=True):
    import concourse.mybir as mybir
    import concourse.tile as tile
    from concourse import bacc

    F32 = mybir.dt.float32
    F32R = mybir.dt.float32r
    BF16 = mybir.dt.bfloat16
    F16 = mybir.dt.float16
    AF = mybir.ActivationFunctionType
    OP = mybir.AluOpType

    nc = bacc.Bacc("TRN2", target_bir_lowering=False, debug=False,
                   num_devices=NCORES)

    # ---- DRAM I/O ----
    featsT_d = nc.dram_tensor("featsT", [E, R, BL], F32R, kind="ExternalInput")
    featsb_d = nc.dram_tensor("featsb", [E, BL, R], BF16, kind="ExternalInput")
    embT_d = nc.dram_tensor("embT", [E, T * BL], F32R, kind="ExternalInput")
    attn_We_d = nc.dram_tensor("attn_We", [E, H], F32R, kind="ExternalInput")
    attn_Wh_d = nc.dram_tensor("attn_Wh", [H, H], BF16, kind="ExternalInput")
    W_hhT_d = nc.dram_tensor("W_hhT", [H, 3 * H], BF16, kind="ExternalInput")
    W_ihcT_d = nc.dram_tensor("W_ihcT", [E, 3 * H], BF16, kind="ExternalInput")
    W_iheT_d = nc.dram_tensor("W_iheT", [E, 3 * H], F32R, kind="ExternalInput")
    vw_d = nc.dram_tensor("vw", [H, 1], BF16, kind="ExternalInput")
    bsum_d = nc.dram_tensor("bsum", [3 * H, 1], F32, kind="ExternalInput")
    attnb_d = nc.dram_tensor("attnb", [H, 1], F32, kind="ExternalInput")
    fcW_d = nc.dram_tensor("fcW", [H, V], F16, kind="ExternalInput")
    out_d = nc.dram_tensor("out", [T * BL, V], F32, kind="ExternalOutput")

    r3 = lambda ap: ap.rearrange("(kt p) m -> p kt m", p=128)

    with tile.TileContext(nc) as tc:
        with tc.tile_pool(name="persist", bufs=1) as P1:
            # ---- resident tensors (recurrence weights on the Pool queue) ----
            feats_bf = P1.tile([128, KT, BL, R], BF16)
            nc.gpsimd.dma_start(feats_bf[:], featsb_d.ap().rearrange(
                "(kt p) b r -> p kt b r", p=128))
            attn_Wh = P1.tile([128, KT, H], BF16)
            nc.gpsimd.dma_start(attn_Wh[:], r3(attn_Wh_d.ap()))
            W_hhT = P1.tile([128, KT, 3 * H], BF16)
            nc.gpsimd.dma_start(W_hhT[:], r3(W_hhT_d.ap()))
            W_ihcT = P1.tile([128, KT, 3 * H], BF16)
            nc.gpsimd.dma_start(W_ihcT[:], r3(W_ihcT_d.ap()))
            vw = P1.tile([128, KT, 1], BF16)
            nc.gpsimd.dma_start(vw[:], r3(vw_d.ap()))
            bsum = P1.tile([128, M3H, 1], F32)
            nc.gpsimd.dma_start(bsum[:], r3(bsum_d.ap()))
            attnb = P1.tile([128, KT, 1], F32)
            nc.gpsimd.dma_start(attnb[:], r3(attnb_d.ap()))
            ones_row = P1.tile([1, 128], F32)
            nc.vector.memset(ones_row[:], 1.0)
            ones_b = P1.tile([1, 128], BF16)
            nc.vector.memset(ones_b[:], 1.0)
            # fc weights tile (DMA issued after precompute, below)
            fcW = P1.tile([128, KT, V], F16)
            # fp16 hidden-state history (columns t*BL+b), filled per step
            h_all = P1.tile([128, KT, T * BL], F16)
            # feat_proj (tanh-input bias from features), filled below
            fpT = P1.tile([128, KT, R, BL], BF16)
            # xgx: embedding side of gate preactivations + biases
            xgxT = P1.tile([128, M3H, T * BL], F32)

            with tc.tile_pool(name="pre", bufs=1) as PP, \
                 tc.tile_pool(name="pre_ps", bufs=2, space="PSUM") as PPS:
                # feat_proj = features @ attn_W[:E] + attn_b  (feature-major)
                featsT = PP.tile([128, KT, R, BL], F32R)
                nc.sync.dma_start(featsT[:], featsT_d.ap().rearrange(
                    "(kt p) r b -> p kt r b", p=128))
                attn_We = PP.tile([128, KT, H], F32R)
                nc.sync.dma_start(attn_We[:], r3(attn_We_d.ap()))
                for mo in range(KT):
                    ps = PPS.tile([128, R * BL], F32, name="fp_ps")
                    for kt in range(KT):
                        nc.tensor.matmul(
                            ps[:], attn_We[:, kt, mo * 128:(mo + 1) * 128],
                            featsT[:, kt].rearrange("p r b -> p (r b)"),
                            start=(kt == 0), stop=(kt == KT - 1))
                    nc.vector.tensor_scalar(
                        out=fpT[:, mo].rearrange("p r b -> p (r b)"),
                        in0=ps[:], scalar1=attnb[:, mo], scalar2=None,
                        op0=OP.add)
                # xgx = emb @ W_ih[:, :E].T + (b_ih + b_hh)
                W_iheT = PP.tile([128, KT, 3 * H], F32R)
                nc.scalar.dma_start(W_iheT[:], r3(W_iheT_d.ap()))
                embT = PP.tile([128, KT, T * BL], F32R)
                nc.scalar.dma_start(embT[:], r3(embT_d.ap()))
                for m in range(M3H):
                    ps = PPS.tile([128, T * BL], F32, name="xg_ps")
                    for kt in range(KT):
                        nc.tensor.matmul(
                            ps[:], W_iheT[:, kt, m * 128:(m + 1) * 128],
                            embT[:, kt], start=(kt == 0), stop=(kt == KT - 1))
                    nc.vector.tensor_scalar(
                        out=xgxT[:, m], in0=ps[:], scalar1=bsum[:, m],
                        scalar2=None, op0=OP.add)

            # fc weight prefetch: issued after the precompute's input DMAs so
            # those go first in the queue; finishes during the recurrence
            for kt in range(KT):
                nc.sync.dma_start(fcW[:, kt], r3(fcW_d.ap())[:, kt])

            # ---- recurrence ----
            with tc.tile_pool(name="state", bufs=2) as PST, \
                 tc.tile_pool(name="scratch", bufs=2) as PSC, \
                 tc.tile_pool(name="gates", bufs=2) as PG, \
                 tc.tile_pool(name="ps_hp", bufs=2, space="PSUM") as PS_HP, \
                 tc.tile_pool(name="ps_sc", bufs=2, space="PSUM") as PS_SC, \
                 tc.tile_pool(name="ps_ctx", bufs=2, space="PSUM") as PS_CTX, \
                 tc.tile_pool(name="ps_g", bufs=1, space="PSUM") as PS_G:
                h_T = PST.tile([128, KT, BL], BF16, name="h_init")
                nc.vector.memset(h_T[:], 0.0)

                for t in range(T):
                    # gh = W_hh @ h (fills PE while attention runs)
                    g_gh = PS_G.tile([128, M3H, BL], F32, name="g_gh")
                    g_cgx = PS_G.tile([128, M3H, BL], F32, name="g_cgx")
                    for m in range(M3H):
                        for kt in range(KT):
                            nc.tensor.matmul(
                                g_gh[:, m], W_hhT[:, kt, m * 128:(m + 1) * 128],
                                h_T[:, kt], start=(kt == 0),
                                stop=(kt == KT - 1))

                    xg = xgxT[:, :, t * BL:(t + 1) * BL]

                    # h_proj = attn_W[E:] @ h   (feature-major out)
                    hp = PS_HP.tile([128, KT, BL], F32, name="hp")
                    for mo in range(KT):
                        for kt in range(KT):
                            nc.tensor.matmul(
                                hp[:, mo], attn_Wh[:, kt, mo * 128:(mo + 1) * 128],
                                h_T[:, kt], start=(kt == 0), stop=(kt == KT - 1))

                    # energy = tanh(feat_proj + h_proj); scores = v_w . energy
                    # pipelined in two r-halves across DVE -> ACT -> PE
                    hp_bf = PSC.tile([128, KT, BL], BF16, name="hp_bf")
                    nc.vector.tensor_copy(hp_bf[:], hp[:])
                    sc = PS_SC.tile([1, R, BL], F32, name="sc", bufs=1)
                    en_b = PSC.tile([128, KT, R, BL], BF16, name="en_b", bufs=1)
                    for (r0, r1) in RSPLIT:
                        nr = r1 - r0
                        en_f = PSC.tile([128, KT, 25, BL], BF16,
                                        name=f"en_f{r0}", bufs=1)
                        nc.vector.tensor_tensor(
                            out=en_f[:, :, :nr], in0=fpT[:, :, r0:r1],
                            in1=hp_bf[:, :, None, :].to_broadcast(
                                (128, KT, nr, BL)),
                            op=OP.add)
                        nc.scalar.activation(
                            en_b[:, :, r0:r1], en_f[:, :, :nr], AF.Tanh)
                        for kt in range(KT):
                            nc.tensor.matmul(
                                sc[:, r0:r1].rearrange("p r b -> p (r b)"),
                                vw[:, kt],
                                en_b[:, kt, r0:r1].rearrange("p r b -> p (r b)"),
                                start=(kt == 0), stop=(kt == KT - 1))

                    # gate pre-add needing only gh + constants: emitted
                    # after the energy chain so DVE prioritizes the chain
                    rzpre = PG.tile([128, 8, BL], F32, name="rzpre")
                    nc.vector.tensor_tensor(
                        out=rzpre[:], in0=g_gh[:, 0:8], in1=xg[:, 0:8],
                        op=OP.add)

                    # softmax, unnormalized (scores are O(1): no max-sub;
                    # the 1/sum lands on the context below). bf16 exp is
                    # replicated across partitions by a PE rank-1 broadcast;
                    # the per-b 1/sum the same way, overlapping each other.
                    ex = PSC.tile([1, BL, R], BF16, name="ex")
                    nc.scalar.activation(
                        ex[:].rearrange("p b r -> p r b"), sc[:], AF.Exp)
                    exb_ps = PS_CTX.tile([128, BL * R], F32, name="exb_ps")
                    nc.tensor.matmul(
                        exb_ps[:], ones_b[:], ex[:].rearrange("p b r -> p (b r)"),
                        start=True, stop=True)
                    exb = PSC.tile([128, BL, R], BF16, name="exb", bufs=1)
                    nc.vector.tensor_copy(
                        exb[:].rearrange("p b r -> p (b r)"), exb_ps[:])
                    # context = sum_r attn * feats (bf16 DVE mult + reduce)
                    prod = PSC.tile([128, KT, BL, R], BF16, name="prod",
                                    bufs=1)
                    nc.vector.tensor_tensor(
                        out=prod[:], in0=feats_bf[:],
                        in1=exb[:, None].to_broadcast((128, KT, BL, R)),
                        op=OP.mult)
                    ctx_u = PSC.tile([128, KT, BL], F32, name="ctx_u")
                    nc.vector.tensor_reduce(
                        out=ctx_u[:], in_=prod[:],
                        axis=mybir.AxisListType.X, op=OP.add)
                    s_sum = PSC.tile([1, BL], F32, name="s_sum")
                    nc.vector.tensor_reduce(
                        out=s_sum[:], in_=ex[:],
                        axis=mybir.AxisListType.X, op=OP.add)
                    rec = PSC.tile([1, BL], F32, name="rec")
                    nc.vector.reciprocal(rec[:], s_sum[:])
                    recb_ps = PS_CTX.tile([128, BL], F32, name="recb_ps",
                                          bufs=1)
                    nc.tensor.matmul(recb_ps[:], ones_row[:], rec[:],
                                     start=True, stop=True)
                    recb = PSC.tile([128, BL], F32, name="recb")
                    nc.vector.tensor_copy(recb[:], recb_ps[:])
                    ctx_bf = PSC.tile([128, KT, BL], BF16, name="ctx_bf")
                    nc.vector.tensor_tensor(
                        out=ctx_bf[:], in0=ctx_u[:],
                        in1=recb[:, None, :].to_broadcast((128, KT, BL)),
                        op=OP.mult)

                    # cgx = W_ih[:, E:] @ context
                    for m in range(M3H):
                        for kt in range(KT):
                            nc.tensor.matmul(
                                g_cgx[:, m], W_ihcT[:, kt, m * 128:(m + 1) * 128],
                                ctx_bf[:, kt], start=(kt == 0),
                                stop=(kt == KT - 1))

                    # gates: r,z = 0.5*(1+tanh(0.5*x)); n = tanh(xn + r*hn)
                    xn_tot = PG.tile([128, 4, BL], F32, name="xn_tot")
                    nc.vector.tensor_tensor(
                        out=xn_tot[:], in0=g_cgx[:, 8:12], in1=xg[:, 8:12],
                        op=OP.add)
                    rz_t = PG.tile([128, 8, BL], F32, name="rz_t")
                    nc.vector.tensor_tensor(
                        out=rz_t[:], in0=g_cgx[:, 0:8], in1=rzpre[:],
                        op=OP.add)
                    rz_h = PG.tile([128, 8, BL], F32, name="rz_h")
                    nc.scalar.activation(rz_h[:], rz_t[:], AF.Tanh, scale=0.5)
                    rz = PG.tile([128, 8, BL], F32, name="rz")
                    nc.vector.tensor_scalar(
                        out=rz[:], in0=rz_h[:], scalar1=0.5, scalar2=0.5,
                        op0=OP.mult, op1=OP.add)
                    n_p = PG.tile([128, 4, BL], F32, name="n_p")
                    nc.vector.tensor_tensor(
                        out=n_p[:], in0=rz[:, 0:4], in1=g_gh[:, 8:12],
                        op=OP.mult)
                    nc.vector.tensor_tensor(
                        out=n_p[:], in0=n_p[:], in1=xn_tot[:], op=OP.add)
                    n_t = PG.tile([128, 4, BL], F32, name="n_t")
                    nc.scalar.activation(n_t[:], n_p[:], AF.Tanh)
                    # h_new = n + z*(h - n)
                    hmn = PG.tile([128, 4, BL], F32, name="hmn")
                    nc.vector.tensor_tensor(
                        out=hmn[:], in0=h_T[:], in1=n_t[:], op=OP.subtract)
                    h_new = PST.tile([128, KT, BL], BF16, name="h_new")
                    nc.vector.tensor_tensor(
                        out=hmn[:], in0=rz[:, 4:8], in1=hmn[:], op=OP.mult)
                    nc.vector.tensor_tensor(
                        out=h_new[:], in0=n_t[:], in1=hmn[:], op=OP.add)
                    # fp16 history for the fc matmul
                    nc.vector.tensor_copy(
                        h_all[:, :, t * BL:(t + 1) * BL], h_new[:])
                    h_T = h_new

            # ---- final fc: logits = h_all.T @ fc_W (+ fc_b) ----
            NCH = (V + 511) // 512  # 20 chunks, last = 272
            with tc.tile_pool(name="fc_ps", bufs=4, space="PSUM") as FPS, \
                 tc.tile_pool(name="fc_sb", bufs=4) as FSB, \
                 tc.tile_pool(name="fc_bias", bufs=1) as FB:
                fcb = None
                if has_fcb:
                    fcb_d = nc.dram_tensor("fcb", [1, V], F16,
                                           kind="ExternalInput")
                    fcb = FB.tile([128, V], F16)
                    nc.sync.dma_start(fcb[:], fcb_d.ap().to_broadcast((128, V)))
                for mo in range(2):
                    rows = slice(mo * 128, (mo + 1) * 128)
                    for ch in range(NCH):
                        nv = min(512, V - ch * 512)
                        cols = slice(ch * 512, ch * 512 + nv)
                        ps = FPS.tile([128, 512], F32, name="fc_ps")
                        for kt in range(KT):
                            nc.tensor.matmul(
                                ps[:, :nv], h_all[:, kt, rows],
                                fcW[:, kt, cols], start=(kt == 0),
                                stop=(kt == KT - 1))
                        ot = FSB.tile([128, 512], F32, name="fc_ot")
                        k = (mo * NCH + ch) % 3
                        if has_fcb:
                            nc.vector.tensor_tensor(
                                out=ot[:, :nv], in0=ps[:, :nv],
                                in1=fcb[:, cols], op=OP.add)
                        elif k == 2:
                            nc.scalar.copy(ot[:, :nv], ps[:, :nv])
                        else:
                            nc.vector.tensor_copy(ot[:, :nv], ps[:, :nv])
                        dma_eng = nc.sync if (mo * NCH + ch) % 2 == 0 else nc.scalar
                        dma_eng.dma_start(out_d.ap()[rows, cols], ot[:, :nv])

    nc.compile()
    return nc


def _get_built(has_fcb=True):
    with _BUILD_LOCK:
        if has_fcb not in _BUILT:
            _BUILT[has_fcb] = _build(has_fcb)
    return _BUILT[has_fcb]


def kernel(features, captions, embed_table, attn_W, attn_b, v_w,
           W_ih, W_hh, b_ih, b_hh, fc_W, fc_b):
    from concourse.bass_utils import run_bass_kernel_spmd

    features = np.asarray(features, dtype=np.float32)
    captions = np.asarray(captions)
    embed_table = np.asarray(embed_table, dtype=np.float32)
    attn_W = np.asarray(attn_W, dtype=np.float32)
    attn_b = np.asarray(attn_b, dtype=np.float32)
    v_w = np.asarray(v_w, dtype=np.float32)
    W_ih = np.asarray(W_ih, dtype=np.float32)
    W_hh = np.asarray(W_hh, dtype=np.float32)
    b_ih = np.asarray(b_ih, dtype=np.float32)
    b_hh = np.asarray(b_hh, dtype=np.float32)
    fc_W = np.asarray(fc_W, dtype=np.float32)
    fc_b = np.asarray(fc_b, dtype=np.float32)

    has_fcb = bool(np.any(fc_b))
    nc = _get_built(has_fcb)

    bf16 = ml_dtypes.bfloat16
    shared = {
        "attn_We": _round_f32r(attn_W[:E]),
        "attn_Wh": attn_W[E:].astype(bf16),
        "W_hhT": np.ascontiguousarray(W_hh.T).astype(bf16),
        "W_ihcT": np.ascontiguousarray(W_ih[:, E:].T).astype(bf16),
        "W_iheT": _round_f32r(W_ih[:, :E].T),
        "vw": v_w[:, None].astype(bf16),
        "bsum": np.ascontiguousarray((b_ih + b_hh)[:, None]),
        "attnb": np.ascontiguousarray(attn_b[:, None]),
        "fcW": fc_W.astype(np.float16),
    }
    if has_fcb:
        shared["fcb"] = fc_b[None, :].astype(np.float16)
    emb = embed_table[captions[:, :T].astype(np.int64)]  # [B, T, E]
    in_maps = []
    for c in range(NCORES):
        rows = slice(c * BL, (c + 1) * BL)
        m = dict(shared)
        m["featsT"] = _round_f32r(features[rows].transpose(2, 1, 0))
        m["featsb"] = features[rows].transpose(2, 0, 1).astype(bf16)
        m["embT"] = _round_f32r(
            emb[rows].transpose(2, 1, 0).reshape(E, T * BL))
        in_maps.append(m)

    res = run_bass_kernel_spmd(nc, in_maps, core_ids=list(range(NCORES)))

    out = np.empty((B, T, V), dtype=np.float32)
    for c in range(NCORES):
        # rows of per-core output are t*BL + b_local
        out[c * BL:(c + 1) * BL] = (
            res.results[c]["out"].reshape(T, BL, V).transpose(1, 0, 2))
    return out



# revision 32
# speedup vs baseline: 1.4381x; 1.4381x over previous
"""Trainium2 Bass kernel for nn_DecoderGRU (attention GRU decoder + vocab head).

Strategy (8 NeuronCores, data-parallel over batch, 8 rows/core):
  - Feature-major layouts; fp16 for every PE operand.
  - Per step the serial chain is: hp (PE) -> energy add (DVE, r-halves) ->
    tanh (ACT) -> scores (PE, b-outer padded psum) -> psum->sbuf copy (DVE)
    -> 4 K=1 transpose matmuls (PE) -> exp (ACT, [128,4] psum) -> context
    as feats-stationary r-contraction matmuls (PE, lo/hi row groups) +
    all-ones matmul for the replicated softmax denominator -> reciprocal
    (DVE) -> normalize fused into ctx psum->sbuf copy -> cgx accumulating
    onto the gate psum (which already holds xg+bias+gh from earlier
    matmuls) -> gate tail (tanh-form sigmoid, exp/tanh stay on one ACT
    table set).
  - fc logits: rows 0-127 interleaved into PE idle windows of steps 16+,
    rows 128-255 at the end.
"""

import threading

import numpy as np
import ml_dtypes

B, R, E, H, V, L = 64, 49, 512, 512, 10000, 33
T = L - 1            # 32 decode steps
NCORES = 8
BL = B // NCORES     # 8 batch rows per core
KT = E // 128        # 4 k-tiles of 128 for E=H=512
M3H = (3 * H) // 128  # 12 m-tiles for gate dim
RH1, RH2 = 25, 24    # r-halves for the energy pipeline
RP = 64              # padded r stride in the score tile
# slot i on device holds batch row PERM[i] of the core's 8 rows
PERM = [0, 2, 4, 6, 1, 3, 5, 7]
NCH = (V + 511) // 512  # 20 vocab chunks of <=512

_BUILD_LOCK = threading.Lock()
_BUILT = {}


def _round_f32r(a):
    """fp32r rounding (drop 13 low mantissa bits, round-to-nearest) on host."""
    v = np.ascontiguousarray(a, dtype=np.float32).view(np.uint32).astype(np.uint64)
    v = (v + 0x1000) & 0xFFFFE000
    return v.astype(np.uint32).view(np.float32)


def _build(has_fcb=True):
    import concourse.mybir as mybir
    import concourse.tile as tile
    from concourse import bacc

    F32 = mybir.dt.float32
    F32R = mybir.dt.float32r
    F16 = mybir.dt.float16
    AF = mybir.ActivationFunctionType
    OP = mybir.AluOpType

    nc = bacc.Bacc("TRN2", target_bir_lowering=False, debug=False,
                   num_devices=NCORES)

    # ---- DRAM I/O (all activation-side tensors already in slot order) ----
    featsT_d = nc.dram_tensor("featsT", [E, R, BL], F32R, kind="ExternalInput")
    featsR_d = nc.dram_tensor("featsR", [128, BL // 2, E], F16,
                              kind="ExternalInput")
    embT_d = nc.dram_tensor("embT", [E, T * BL], F32R, kind="ExternalInput")
    attn_We_d = nc.dram_tensor("attn_We", [E, H], F32R, kind="ExternalInput")
    attn_Wh_d = nc.dram_tensor("attn_Wh", [H, H], F16, kind="ExternalInput")
    W_hhT_d = nc.dram_tensor("W_hhT", [H, 2 * H], F16, kind="ExternalInput")
    W_hhn2_d = nc.dram_tensor("W_hhn2", [H, H], F16, kind="ExternalInput")
    W_ihcT_d = nc.dram_tensor("W_ihcT", [E, 3 * H], F16, kind="ExternalInput")
    W_iheT_d = nc.dram_tensor("W_iheT", [E, 3 * H], F32R, kind="ExternalInput")
    vw_d = nc.dram_tensor("vw", [H, 1], F16, kind="ExternalInput")
    bsum_d = nc.dram_tensor("bsum", [1, 3 * H], F16, kind="ExternalInput")
    attnb_d = nc.dram_tensor("attnb", [H, 1], F32, kind="ExternalInput")
    fcW_d = nc.dram_tensor("fcW", [H, V], F16, kind="ExternalInput")
    out_d = nc.dram_tensor("out", [T * BL, V], F32, kind="ExternalOutput")

    r3 = lambda ap: ap.rearrange("(kt p) m -> p kt m", p=128)

    with tile.TileContext(nc) as tc:
        with tc.tile_pool(name="persist", bufs=1) as P1:
            # ---- resident tensors ----
            attn_Wh = P1.tile([128, KT, H], F16)
            nc.gpsimd.dma_start(attn_Wh[:], r3(attn_Wh_d.ap()))
            W_hhT = P1.tile([128, KT, 2 * H], F16)
            nc.gpsimd.dma_start(W_hhT[:], r3(W_hhT_d.ap()))
            W_hhn2 = P1.tile([128, KT, H], F16)
            nc.gpsimd.dma_start(W_hhn2[:], r3(W_hhn2_d.ap()))
            W_ihcT = P1.tile([128, KT, 3 * H], F16)
            nc.gpsimd.dma_start(W_ihcT[:], r3(W_ihcT_d.ap()))
            W_iheT = P1.tile([128, KT, 3 * H], F32R)
            nc.gpsimd.dma_start(W_iheT[:], r3(W_iheT_d.ap()))
            embT = P1.tile([128, KT, T * BL], F32R)
            nc.scalar.dma_start(embT[:], r3(embT_d.ap()))
            vw = P1.tile([128, KT, 1], F16)
            nc.gpsimd.dma_start(vw[:], r3(vw_d.ap()))
            bsum = P1.tile([1, M3H, 128], F16)
            nc.gpsimd.dma_start(bsum[:], bsum_d.ap().rearrange(
                "a (m p) -> a m p", p=128))
            attnb = P1.tile([128, KT, 1], F32)
            nc.gpsimd.dma_start(attnb[:], r3(attnb_d.ap()))
            featsR = P1.tile([128, BL // 2, E], F16)
            nc.gpsimd.dma_start(featsR[:], featsR_d.ap())
            ones128 = P1.tile([128, 128], F16)
            nc.vector.memset(ones128[:], 1.0)
            one1 = P1.tile([1, 1], F16)
            nc.vector.memset(one1[:], 1.0)
            z16 = P1.tile([128, KT, BL], F16)
            nc.vector.memset(z16[:], 0.0)
            # padded b-outer exp row; pad columns stay zero forever
            ex16 = P1.tile([1, BL, RP], F16)
            nc.vector.memset(ex16[:], 0.0)
            # fp16 hidden-state history (columns t*BL+slot)
            h_all = P1.tile([128, KT, T * BL], F16)
            # energy scratch (written every step)
            en16 = P1.tile([128, KT, R, BL], F16)
            enb = P1.tile([128, KT, R, BL], F16)
            # feat_proj + attn_b, fp16 feature-major
            fpT = P1.tile([128, KT, R, BL], F16)
            # fc weights (DMA issued after precompute DMAs)
            fcW = P1.tile([128, KT, V], F16)
            fcb = None
            if has_fcb:
                fcb_d = nc.dram_tensor("fcb", [1, V], F16,
                                       kind="ExternalInput")
                fcb = P1.tile([128, V], F16)
                nc.scalar.dma_start(fcb[:], fcb_d.ap().to_broadcast((128, V)))

            # ---- precompute: feat_proj = feats @ attn_W[:E] + attn_b ----
            with tc.tile_pool(name="pre", bufs=1) as PP, \
                 tc.tile_pool(name="pre_ps", bufs=2, space="PSUM") as PPS:
                featsT = PP.tile([128, KT, R, BL], F32R)
                nc.sync.dma_start(featsT[:], featsT_d.ap().rearrange(
                    "(kt p) r b -> p kt r b", p=128))
                attn_We = PP.tile([128, KT, H], F32R)
                nc.sync.dma_start(attn_We[:], r3(attn_We_d.ap()))
                for mo in range(KT):
                    ps = PPS.tile([128, R * BL], F32, name="fp_ps")
                    for kt in range(KT):
                        nc.tensor.matmul(
                            ps[:], attn_We[:, kt, mo * 128:(mo + 1) * 128],
                            featsT[:, kt].rearrange("p r b -> p (r b)"),
                            start=(kt == 0), stop=(kt == KT - 1))
                    nc.vector.tensor_scalar(
                        out=fpT[:, mo].rearrange("p r b -> p (r b)"),
                        in0=ps[:], scalar1=attnb[:, mo], scalar2=None,
                        op0=OP.add)

            # fc weights after precompute DMAs; finishes during recurrence
            for kt in range(KT):
                nc.sync.dma_start(fcW[:, kt], r3(fcW_d.ap())[:, kt])

            # ---- recurrence ----
            with tc.tile_pool(name="st", bufs=2) as PST, \
                 tc.tile_pool(name="ps_misc", bufs=1, space="PSUM") as PS_M, \
                 tc.tile_pool(name="ps_sc", bufs=1, space="PSUM") as PS_SC, \
                 tc.tile_pool(name="ps_g", bufs=2, space="PSUM") as PS_G, \
                 tc.tile_pool(name="ps_fc", bufs=1, space="PSUM") as PS_FC, \
                 tc.tile_pool(name="fc_sb", bufs=2) as FSB:
                # fc pass-1 schedule: one 2-chunk unit per step from step 16;
                # its psum->sbuf copy runs on DVE early in the NEXT step
                fc1 = {16 + i: (2 * i, min(2 * i + 2, NCH))
                       for i in range((NCH + 1) // 2)}
                fc_pending = None  # (fps, ot, cols, nv) awaiting copy+DMA

                for t in range(T):
                    h_prev = (h_all[:, :, (t - 1) * BL:t * BL] if t > 0
                              else z16[:])

                    # gate psum bank: one long accumulation group
                    # (xg+bias+gh+hn+cgx); only the first matmul starts it
                    # and only the last cgx matmul stops it (one group/bank)
                    gbank = PS_G.tile([128, 512], F32, name="gbank")
                    ghx = gbank[:, 0:M3H * BL].rearrange(
                        "p (m b) -> p m b", b=BL)
                    hn_ps = gbank[:, M3H * BL:(M3H + KT) * BL].rearrange(
                        "p (m b) -> p m b", b=BL)
                    # misc psum bank: hp, ctx, scT, srep; their groups open
                    # and close strictly in program order
                    mbank = PS_M.tile([128, 512], F32, name="mbank")
                    hp = mbank[:, 0:KT * BL].rearrange(
                        "p (k b) -> p k b", b=BL)
                    ctx_ps = mbank[:, KT * BL:2 * KT * BL].rearrange(
                        "p (k b) -> p k b", b=BL)
                    scT = mbank[:, 2 * KT * BL:2 * KT * BL + 4]
                    srep = mbank[:, 2 * KT * BL + 4:2 * KT * BL + 12]
                    emb_t = embT[:, :, t * BL:(t + 1) * BL]
                    for m in range(M3H):
                        for kt in range(KT):
                            nc.tensor.matmul(
                                ghx[:, m], W_iheT[:, kt, m * 128:(m + 1) * 128],
                                emb_t[:, kt], start=(m == 0 and kt == 0),
                                stop=False)
                        nc.tensor.matmul(
                            ghx[:, m], bsum[:, m], ones128[0:1, 0:BL],
                            start=False, stop=False)
                    # hp = attn_Wh @ h  (head of the chain)
                    for mo in range(KT):
                        for kt in range(KT):
                            nc.tensor.matmul(
                                hp[:, mo], attn_Wh[:, kt, mo * 128:(mo + 1) * 128],
                                h_prev[:, kt], start=(kt == 0),
                                stop=(kt == KT - 1))
                    # gh: r,z rows into ghx; n rows at half strength go into
                    # BOTH hn (for r*hn) and ghx (r*hn = hn/2 + th_r*hn/2)
                    for m in range(8):
                        for kt in range(KT):
                            nc.tensor.matmul(
                                ghx[:, m], W_hhT[:, kt, m * 128:(m + 1) * 128],
                                h_prev[:, kt], start=False, stop=False)
                    for m in range(4):
                        for kt in range(KT):
                            nc.tensor.matmul(
                                hn_ps[:, m], W_hhn2[:, kt, m * 128:(m + 1) * 128],
                                h_prev[:, kt], start=False, stop=False)
                        for kt in range(KT):
                            nc.tensor.matmul(
                                ghx[:, 8 + m], W_hhn2[:, kt, m * 128:(m + 1) * 128],
                                h_prev[:, kt], start=False, stop=False)

                    # energy = tanh(fp + hp), pipelined in two r-halves
                    hp16 = PST.tile([128, KT, BL], F16, name="hp16")
                    nc.vector.tensor_copy(hp16[:], hp[:])
                    # separate psum tiles per r-half so the first copy does
                    # not wait on the second half's matmuls (bank-level deps)
                    sc_psA = PS_SC.tile([1, BL, RH1], F32, name="sc_psA")
                    sc_psB = PS_SC.tile([1, BL, RH2], F32, name="sc_psB")
                    halves = ((0, RH1, sc_psA), (RH1, R, sc_psB))
                    for (r0, r1, sps) in halves:
                        nr = r1 - r0
                        nc.vector.tensor_tensor(
                            out=en16[:, :, r0:r1], in0=fpT[:, :, r0:r1],
                            in1=hp16[:, :, None, :].to_broadcast(
                                (128, KT, nr, BL)),
                            op=OP.add)
                        nc.scalar.activation(
                            enb[:, :, r0:r1], en16[:, :, r0:r1], AF.Tanh)
                        for kt in range(KT):
                            nc.tensor.matmul(
                                sps[:].rearrange("p b r -> p (b r)"),
                                vw[:, kt],
                                enb[:, kt, r0:r1, :].rearrange("p r b -> p b r"),
                                start=(kt == 0), stop=(kt == KT - 1))
                    # exp straight off the score psum (first half overlaps
                    # the second half's matmuls on the PE)
                    for (r0, r1, sps) in halves:
                        nc.scalar.activation(ex16[:, :, r0:r1], sps[:], AF.Exp)

                    # fc pass-1 copy for the previous step's unit: split into
                    # four low-priority pieces so the scheduler slots them
                    # into idle DVE time without blocking the chain
                    if fc_pending is not None:
                        pfps, pot, pcols, pnv = fc_pending
                        with tc.high_priority(offset=-(10 ** 6)):
                            for qi in range(4):
                                a = qi * 256
                                b = min((qi + 1) * 256, pnv)
                                if a >= b:
                                    continue
                                if has_fcb:
                                    nc.vector.tensor_tensor(
                                        out=pot[:, a:b], in0=pfps[:, a:b],
                                        in1=fcb[:, pcols.start + a:
                                                pcols.start + b], op=OP.add)
                                else:
                                    nc.vector.tensor_copy(
                                        pot[:, a:b], pfps[:, a:b])
                            dma_eng = nc.sync if t % 2 == 0 else nc.scalar
                            dma_eng.dma_start(out_d.ap()[0:128, pcols],
                                              pot[:, :pnv])
                        fc_pending = None

                    # transpose exp(scores) to r-on-partitions: 4 K=1 matmuls
                    exflat = ex16[:].rearrange("p b r -> p (b r)")
                    for c in range(BL // 2):
                        nc.tensor.matmul(
                            scT[:, c:c + 1], exflat[:, c * 128:(c + 1) * 128],
                            one1[:], start=True, stop=True)
                    exTs = PST.tile([128, BL // 2], F16, name="exTs")
                    nc.vector.tensor_copy(exTs[:], scT[:])

                    # replicated softmax denominator (before ctx so the
                    # reciprocal overlaps the ctx matmuls), then context
                    nc.tensor.matmul(srep[:, 0:4], ones128[0:49, :],
                                     exTs[0:49, :], start=True, stop=True)
                    nc.tensor.matmul(srep[:, 4:8], ones128[64:113, :],
                                     exTs[64:113, :], start=True, stop=True)
                    for c in range(BL // 2):
                        for k in range(KT):
                            nc.tensor.matmul(
                                ctx_ps[:, k, 2 * c:2 * c + 1],
                                featsR[0:49, c, k * 128:(k + 1) * 128],
                                exTs[0:49, c:c + 1], start=True, stop=True)
                            nc.tensor.matmul(
                                ctx_ps[:, k, 2 * c + 1:2 * c + 2],
                                featsR[64:113, c, k * 128:(k + 1) * 128],
                                exTs[64:113, c:c + 1], start=True, stop=True)
                    # srep cols: [s0 s2 s4 s6 | s1 s3 s5 s7] (slot parity)
                    rec = PST.tile([128, BL], F32, name="rec")
                    nc.vector.reciprocal(rec[:], srep[:])
                    ctx16 = PST.tile([128, KT, BL], F16, name="ctx16")
                    nc.vector.tensor_tensor(
                        out=ctx16[:].rearrange("p k (j par) -> p k j par",
                                               par=2),
                        in0=ctx_ps[:].rearrange("p k (j par) -> p k j par",
                                                par=2),
                        in1=rec[:].rearrange("p (par j) -> p j par", par=2)[
                            :, None, :, :].to_broadcast((128, KT, 4, 2)),
                        op=OP.mult)

                    # cgx accumulates onto ghx; the very last matmul closes
                    # the bank's accumulation group
                    for m in range(M3H):
                        for kt in range(KT):
                            nc.tensor.matmul(
                                ghx[:, m], W_ihcT[:, kt, m * 128:(m + 1) * 128],
                                ctx16[:, kt], start=False,
                                stop=(m == M3H - 1 and kt == KT - 1))

                    # fc interleave: rows 0-127 during steps 16+; matmuls
                    # here (PE idles during the gate tail), copy next step
                    if t in fc1:
                        c0, c1 = fc1[t]
                        nv = min(512 * c1, V) - 512 * c0
                        cols = slice(c0 * 512, c0 * 512 + nv)
                        fps = PS_FC.tile([128, 1024], F32, name="fc_ps")
                        for ch in range(c0, c1):
                            cnv = min(512, V - ch * 512)
                            for kt in range(KT):
                                nc.tensor.matmul(
                                    fps[:, (ch - c0) * 512:(ch - c0) * 512 + cnv],
                                    h_all[:, kt, 0:128],
                                    fcW[:, kt, ch * 512:ch * 512 + cnv],
                                    start=(kt == 0), stop=(kt == KT - 1))
                        ot = FSB.tile([128, 1024], F32, name="fc_ot")
                        fc_pending = (fps, ot, cols, nv)

                    # gates. r,z = 0.5*(1+tanh(0.5*x)). ghx n-rows already
                    # hold xn + hn/2 + cgx_n, and hn_ps holds hn/2, so
                    # n_pre = ghx_n + tanh(0.5*x_r)*hn/2 with no affine fix.
                    rz_h = PST.tile([128, 8, BL], F32, name="rz_h")
                    nc.scalar.activation(rz_h[:], ghx[:, 0:8], AF.Tanh,
                                         scale=0.5)
                    npre = PST.tile([128, KT, BL], F32, name="npre")
                    t_r = PST.tile([128, KT, BL], F32, name="t_r")
                    nc.vector.tensor_tensor(
                        out=t_r[:], in0=rz_h[:, 0:4], in1=hn_ps[:], op=OP.mult)
                    nc.vector.tensor_tensor(
                        out=npre[:], in0=ghx[:, 8:12], in1=t_r[:], op=OP.add)
                    # z and 1-z, and z*h ahead of the final tanh
                    z_g = PST.tile([128, KT, BL], F32, name="z_g")
                    zc_g = PST.tile([128, KT, BL], F32, name="zc_g")
                    nc.vector.tensor_scalar(
                        out=z_g[:], in0=rz_h[:, 4:8], scalar1=0.5,
                        scalar2=0.5, op0=OP.mult, op1=OP.add)
                    nc.vector.tensor_scalar(
                        out=zc_g[:], in0=rz_h[:, 4:8], scalar1=-0.5,
                        scalar2=0.5, op0=OP.mult, op1=OP.add)
                    q_g = PST.tile([128, KT, BL], F32, name="q_g")
                    nc.vector.tensor_tensor(
                        out=q_g[:], in0=z_g[:], in1=h_prev, op=OP.mult)
                    n_t = PST.tile([128, KT, BL], F32, name="n_t")
                    nc.scalar.activation(n_t[:], npre[:], AF.Tanh)
                    # h_new = z*h + (1-z)*n, written straight into h_all
                    w_g = PST.tile([128, KT, BL], F32, name="w_g")
                    nc.vector.tensor_tensor(
                        out=w_g[:], in0=zc_g[:], in1=n_t[:], op=OP.mult)
                    nc.vector.tensor_tensor(
                        out=h_all[:, :, t * BL:(t + 1) * BL], in0=q_g[:],
                        in1=w_g[:], op=OP.add)

            # ---- fc pass 2: rows 128-255 ----
            with tc.tile_pool(name="fc2_ps", bufs=4, space="PSUM") as FPS2, \
                 tc.tile_pool(name="fc2_sb", bufs=4) as FSB2:
                for ch in range(NCH):
                    nv = min(512, V - ch * 512)
                    cols = slice(ch * 512, ch * 512 + nv)
                    ps = FPS2.tile([128, 512], F32, name="fc2_ps")
                    for kt in range(KT):
                        nc.tensor.matmul(
                            ps[:, :nv], h_all[:, kt, 128:256],
                            fcW[:, kt, cols], start=(kt == 0),
                            stop=(kt == KT - 1))
                    ot = FSB2.tile([128, 512], F32, name="fc2_ot")
                    if has_fcb:
                        nc.vector.tensor_tensor(
                            out=ot[:, :nv], in0=ps[:, :nv],
                            in1=fcb[:, cols], op=OP.add)
                    elif ch % 2 == 0:
                        nc.vector.tensor_copy(ot[:, :nv], ps[:, :nv])
                    else:
                        nc.scalar.copy(ot[:, :nv], ps[:, :nv])
                    dma_eng = nc.sync if ch % 2 == 0 else nc.scalar
                    dma_eng.dma_start(out_d.ap()[128:256, cols], ot[:, :nv])

    nc.compile()
    return nc


def _get_built(has_fcb=True):
    with _BUILD_LOCK:
        if has_fcb not in _BUILT:
            _BUILT[has_fcb] = _build(has_fcb)
    return _BUILT[has_fcb]


def kernel(features, captions, embed_table, attn_W, attn_b, v_w,
           W_ih, W_hh, b_ih, b_hh, fc_W, fc_b):
    from concourse.bass_utils import run_bass_kernel_spmd

    features = np.asarray(features, dtype=np.float32)
    captions = np.asarray(captions)
    embed_table = np.asarray(embed_table, dtype=np.float32)
    attn_W = np.asarray(attn_W, dtype=np.float32)
    attn_b = np.asarray(attn_b, dtype=np.float32)
    v_w = np.asarray(v_w, dtype=np.float32)
    W_ih = np.asarray(W_ih, dtype=np.float32)
    W_hh = np.asarray(W_hh, dtype=np.float32)
    b_ih = np.asarray(b_ih, dtype=np.float32)
    b_hh = np.asarray(b_hh, dtype=np.float32)
    fc_W = np.asarray(fc_W, dtype=np.float32)
    fc_b = np.asarray(fc_b, dtype=np.float32)

    has_fcb = bool(np.any(fc_b))
    nc = _get_built(has_fcb)

    f16 = np.float16
    shared = {
        "attn_We": _round_f32r(attn_W[:E]),
        "attn_Wh": attn_W[E:].astype(f16),
        "W_hhT": np.ascontiguousarray(W_hh[:2 * H].T).astype(f16),
        "W_hhn2": np.ascontiguousarray(0.5 * W_hh[2 * H:].T).astype(f16),
        "W_ihcT": np.ascontiguousarray(W_ih[:, E:].T).astype(f16),
        "W_iheT": _round_f32r(W_ih[:, :E].T),
        "vw": v_w[:, None].astype(f16),
        "bsum": (b_ih + b_hh)[None, :].astype(f16),
        "attnb": np.ascontiguousarray(attn_b[:, None]),
        "fcW": fc_W.astype(f16),
    }
    if has_fcb:
        shared["fcb"] = fc_b[None, :].astype(f16)
    emb = embed_table[captions[:, :T].astype(np.int64)]  # [B, T, E]
    perm = np.array(PERM)
    in_maps = []
    for c in range(NCORES):
        rows = c * BL + perm             # batch rows in slot order
        fr = features[rows]              # [BL, R, E] slot-ordered
        featsR = np.zeros((128, BL // 2, E), dtype=np.float32)
        for cc in range(BL // 2):
            featsR[0:49, cc] = fr[2 * cc]
            featsR[64:113, cc] = fr[2 * cc + 1]
        m = dict(shared)
        m["featsT"] = _round_f32r(fr.transpose(2, 1, 0))
        m["featsR"] = featsR.astype(f16)
        m["embT"] = _round_f32r(
            emb[rows].transpose(2, 1, 0).reshape(E, T * BL))
        in_maps.append(m)

    res = run_bass_kernel_spmd(nc, in_maps, core_ids=list(range(NCORES)))

    out = np.empty((B, T, V), dtype=np.float32)
    for c in range(NCORES):
        # per-core output rows are t*BL + slot; slot i is batch PERM[i]
        r = res.results[c]["out"].reshape(T, BL, V)
        out[c * BL + perm] = r.transpose(1, 0, 2)
    return out


# revision 36
# speedup vs baseline: 1.4421x; 1.0028x over previous
"""Trainium2 Bass kernel for nn_DecoderGRU (attention GRU decoder + vocab head).

Strategy (8 NeuronCores, data-parallel over batch, 8 rows/core):
  - Feature-major layouts; fp16 for every PE operand.
  - Per step the serial chain is: hp (PE) -> energy add (DVE, r-halves) ->
    tanh (ACT) -> scores (PE, b-outer padded psum) -> psum->sbuf copy (DVE)
    -> 4 K=1 transpose matmuls (PE) -> exp (ACT, [128,4] psum) -> context
    as feats-stationary r-contraction matmuls (PE, lo/hi row groups) +
    all-ones matmul for the replicated softmax denominator -> reciprocal
    (DVE) -> normalize fused into ctx psum->sbuf copy -> cgx accumulating
    onto the gate psum (which already holds xg+bias+gh from earlier
    matmuls) -> gate tail (tanh-form sigmoid, exp/tanh stay on one ACT
    table set).
  - fc logits: rows 0-127 interleaved into PE idle windows of steps 16+,
    rows 128-255 at the end.
"""

import threading

import numpy as np
import ml_dtypes

B, R, E, H, V, L = 64, 49, 512, 512, 10000, 33
T = L - 1            # 32 decode steps
NCORES = 8
BL = B // NCORES     # 8 batch rows per core
KT = E // 128        # 4 k-tiles of 128 for E=H=512
M3H = (3 * H) // 128  # 12 m-tiles for gate dim
RH1, RH2 = 25, 24    # r-halves for the energy pipeline
RP = 64              # padded r stride in the score tile
# slot i on device holds batch row PERM[i] of the core's 8 rows
PERM = [0, 2, 4, 6, 1, 3, 5, 7]
NCH = (V + 511) // 512  # 20 vocab chunks of <=512

_BUILD_LOCK = threading.Lock()
_BUILT = {}


def _round_f32r(a):
    """fp32r rounding (drop 13 low mantissa bits, round-to-nearest) on host."""
    v = np.ascontiguousarray(a, dtype=np.float32).view(np.uint32).astype(np.uint64)
    v = (v + 0x1000) & 0xFFFFE000
    return v.astype(np.uint32).view(np.float32)


def _build(has_fcb=True):
    import concourse.mybir as mybir
    import concourse.tile as tile
    from concourse import bacc

    F32 = mybir.dt.float32
    F32R = mybir.dt.float32r
    F16 = mybir.dt.float16
    AF = mybir.ActivationFunctionType
    OP = mybir.AluOpType

    nc = bacc.Bacc("TRN2", target_bir_lowering=False, debug=False,
                   num_devices=NCORES)

    # ---- DRAM I/O (all activation-side tensors already in slot order) ----
    featsT_d = nc.dram_tensor("featsT", [E, R, BL], F32R, kind="ExternalInput")
    featsR_d = nc.dram_tensor("featsR", [128, BL // 2, E], F16,
                              kind="ExternalInput")
    embT_d = nc.dram_tensor("embT", [E, T * BL], F32R, kind="ExternalInput")
    attn_We_d = nc.dram_tensor("attn_We", [E, H], F32R, kind="ExternalInput")
    attn_Wh_d = nc.dram_tensor("attn_Wh", [H, H], F16, kind="ExternalInput")
    W_hhT_d = nc.dram_tensor("W_hhT", [H, 2 * H], F16, kind="ExternalInput")
    W_hhn2_d = nc.dram_tensor("W_hhn2", [H, H], F16, kind="ExternalInput")
    W_ihcT_d = nc.dram_tensor("W_ihcT", [E, 3 * H], F16, kind="ExternalInput")
    W_iheT_d = nc.dram_tensor("W_iheT", [E, 3 * H], F32R, kind="ExternalInput")
    vw_d = nc.dram_tensor("vw", [H, 1], F16, kind="ExternalInput")
    bsum_d = nc.dram_tensor("bsum", [1, 3 * H], F16, kind="ExternalInput")
    attnb_d = nc.dram_tensor("attnb", [H, 1], F32, kind="ExternalInput")
    fcW_d = nc.dram_tensor("fcW", [H, V], F16, kind="ExternalInput")
    out_d = nc.dram_tensor("out", [T * BL, V], F32, kind="ExternalOutput")

    r3 = lambda ap: ap.rearrange("(kt p) m -> p kt m", p=128)

    with tile.TileContext(nc) as tc:
        with tc.tile_pool(name="persist", bufs=1) as P1:
            # ---- resident tensors ----
            attn_Wh = P1.tile([128, KT, H], F16)
            nc.gpsimd.dma_start(attn_Wh[:], r3(attn_Wh_d.ap()))
            W_hhT = P1.tile([128, KT, 2 * H], F16)
            nc.gpsimd.dma_start(W_hhT[:], r3(W_hhT_d.ap()))
            W_hhn2 = P1.tile([128, KT, H], F16)
            nc.gpsimd.dma_start(W_hhn2[:], r3(W_hhn2_d.ap()))
            W_ihcT = P1.tile([128, KT, 3 * H], F16)
            nc.gpsimd.dma_start(W_ihcT[:], r3(W_ihcT_d.ap()))
            W_iheT = P1.tile([128, KT, 3 * H], F32R)
            nc.gpsimd.dma_start(W_iheT[:], r3(W_iheT_d.ap()))
            embT = P1.tile([128, KT, T * BL], F32R)
            nc.scalar.dma_start(embT[:], r3(embT_d.ap()))
            vw = P1.tile([128, KT, 1], F16)
            nc.gpsimd.dma_start(vw[:], r3(vw_d.ap()))
            bsum = P1.tile([1, M3H, 128], F16)
            nc.gpsimd.dma_start(bsum[:], bsum_d.ap().rearrange(
                "a (m p) -> a m p", p=128))
            attnb = P1.tile([128, KT, 1], F32)
            nc.gpsimd.dma_start(attnb[:], r3(attnb_d.ap()))
            featsR = P1.tile([128, BL // 2, E], F16)
            nc.gpsimd.dma_start(featsR[:], featsR_d.ap())
            ones128 = P1.tile([128, 128], F16)
            nc.vector.memset(ones128[:], 1.0)
            one1 = P1.tile([1, 1], F16)
            nc.vector.memset(one1[:], 1.0)
            z16 = P1.tile([128, KT, BL], F16)
            nc.vector.memset(z16[:], 0.0)
            # padded b-outer exp row; pad columns stay zero forever
            ex16 = P1.tile([1, BL, RP], F16)
            nc.vector.memset(ex16[:], 0.0)
            # fp16 hidden-state history (columns t*BL+slot)
            h_all = P1.tile([128, KT, T * BL], F16)
            # energy scratch (written every step)
            en16 = P1.tile([128, KT, R, BL], F16)
            enb = P1.tile([128, KT, R, BL], F16)
            # feat_proj + attn_b, fp16 feature-major
            fpT = P1.tile([128, KT, R, BL], F16)
            # fc weights (DMA issued after precompute DMAs)
            fcW = P1.tile([128, KT, V], F16)
            fcb = None
            if has_fcb:
                fcb_d = nc.dram_tensor("fcb", [1, V], F16,
                                       kind="ExternalInput")
                fcb = P1.tile([128, V], F16)
                nc.scalar.dma_start(fcb[:], fcb_d.ap().to_broadcast((128, V)))

            # ---- precompute: feat_proj = feats @ attn_W[:E] + attn_b ----
            with tc.tile_pool(name="pre", bufs=1) as PP, \
                 tc.tile_pool(name="pre_ps", bufs=2, space="PSUM") as PPS:
                featsT = PP.tile([128, KT, R, BL], F32R)
                nc.sync.dma_start(featsT[:], featsT_d.ap().rearrange(
                    "(kt p) r b -> p kt r b", p=128))
                attn_We = PP.tile([128, KT, H], F32R)
                nc.sync.dma_start(attn_We[:], r3(attn_We_d.ap()))
                for mo in range(KT):
                    ps = PPS.tile([128, R * BL], F32, name="fp_ps")
                    for kt in range(KT):
                        nc.tensor.matmul(
                            ps[:], attn_We[:, kt, mo * 128:(mo + 1) * 128],
                            featsT[:, kt].rearrange("p r b -> p (r b)"),
                            start=(kt == 0), stop=(kt == KT - 1))
                    nc.vector.tensor_scalar(
                        out=fpT[:, mo].rearrange("p r b -> p (r b)"),
                        in0=ps[:], scalar1=attnb[:, mo], scalar2=None,
                        op0=OP.add)

            # fc weights after precompute DMAs; finishes during recurrence
            for kt in range(KT):
                nc.sync.dma_start(fcW[:, kt], r3(fcW_d.ap())[:, kt])

            # ---- recurrence ----
            with tc.tile_pool(name="st", bufs=2) as PST, \
                 tc.tile_pool(name="ps_misc", bufs=1, space="PSUM") as PS_M, \
                 tc.tile_pool(name="ps_sc", bufs=1, space="PSUM") as PS_SC, \
                 tc.tile_pool(name="ps_g", bufs=2, space="PSUM") as PS_G, \
                 tc.tile_pool(name="ps_fc", bufs=1, space="PSUM") as PS_FC, \
                 tc.tile_pool(name="fc_sb", bufs=2) as FSB:
                # fc pass-1 schedule: one 2-chunk unit per step from step 16;
                # its psum->sbuf copy runs on DVE early in the NEXT step
                fc1 = {16 + i: (2 * i, min(2 * i + 2, NCH))
                       for i in range((NCH + 1) // 2)}
                fc_pending = None  # (fps, ot, cols, nv) awaiting copy+DMA

                for t in range(T):
                    h_prev = (h_all[:, :, (t - 1) * BL:t * BL] if t > 0
                              else z16[:])

                    # gate psum bank: one long accumulation group
                    # (xg+bias+gh+hn+cgx); only the first matmul starts it
                    # and only the last cgx matmul stops it (one group/bank)
                    gbank = PS_G.tile([128, 512], F32, name="gbank")
                    ghx = gbank[:, 0:M3H * BL].rearrange(
                        "p (m b) -> p m b", b=BL)
                    hn_ps = gbank[:, M3H * BL:(M3H + KT) * BL].rearrange(
                        "p (m b) -> p m b", b=BL)
                    # misc psum bank: hp, ctx, scT, srep; their groups open
                    # and close strictly in program order
                    mbank = PS_M.tile([128, 512], F32, name="mbank")
                    hp = mbank[:, 0:KT * BL].rearrange(
                        "p (k b) -> p k b", b=BL)
                    ctx_ps = mbank[:, KT * BL:2 * KT * BL].rearrange(
                        "p (k b) -> p k b", b=BL)
                    scT = mbank[:, 2 * KT * BL:2 * KT * BL + 4]
                    srep = mbank[:, 2 * KT * BL + 4:2 * KT * BL + 12]
                    emb_t = embT[:, :, t * BL:(t + 1) * BL]
                    for m in range(M3H):
                        for kt in range(KT):
                            nc.tensor.matmul(
                                ghx[:, m], W_iheT[:, kt, m * 128:(m + 1) * 128],
                                emb_t[:, kt], start=(m == 0 and kt == 0),
                                stop=False)
                        nc.tensor.matmul(
                            ghx[:, m], bsum[:, m], ones128[0:1, 0:BL],
                            start=False, stop=False)
                    # hp = attn_Wh @ h  (head of the chain)
                    for mo in range(KT):
                        for kt in range(KT):
                            nc.tensor.matmul(
                                hp[:, mo], attn_Wh[:, kt, mo * 128:(mo + 1) * 128],
                                h_prev[:, kt], start=(kt == 0),
                                stop=(kt == KT - 1))
                    # gh: r,z rows into ghx; n rows at half strength go into
                    # BOTH hn (for r*hn) and ghx (r*hn = hn/2 + th_r*hn/2)
                    for m in range(8):
                        for kt in range(KT):
                            nc.tensor.matmul(
                                ghx[:, m], W_hhT[:, kt, m * 128:(m + 1) * 128],
                                h_prev[:, kt], start=False, stop=False)
                    for m in range(4):
                        for kt in range(KT):
                            nc.tensor.matmul(
                                hn_ps[:, m], W_hhn2[:, kt, m * 128:(m + 1) * 128],
                                h_prev[:, kt], start=False, stop=False)
                        for kt in range(KT):
                            nc.tensor.matmul(
                                ghx[:, 8 + m], W_hhn2[:, kt, m * 128:(m + 1) * 128],
                                h_prev[:, kt], start=False, stop=False)

                    # energy = tanh(fp + hp), pipelined in two r-halves
                    hp16 = PST.tile([128, KT, BL], F16, name="hp16")
                    nc.vector.tensor_copy(hp16[:], hp[:])
                    # separate psum tiles per r-half so the first copy does
                    # not wait on the second half's matmuls (bank-level deps)
                    sc_psA = PS_SC.tile([1, BL, RH1], F32, name="sc_psA")
                    sc_psB = PS_SC.tile([1, BL, RH2], F32, name="sc_psB")
                    halves = ((0, RH1, sc_psA), (RH1, R, sc_psB))
                    for (r0, r1, sps) in halves:
                        nr = r1 - r0
                        nc.vector.tensor_tensor(
                            out=en16[:, :, r0:r1], in0=fpT[:, :, r0:r1],
                            in1=hp16[:, :, None, :].to_broadcast(
                                (128, KT, nr, BL)),
                            op=OP.add)
                        nc.scalar.activation(
                            enb[:, :, r0:r1], en16[:, :, r0:r1], AF.Tanh)
                        for kt in range(KT):
                            nc.tensor.matmul(
                                sps[:].rearrange("p b r -> p (b r)"),
                                vw[:, kt],
                                enb[:, kt, r0:r1, :].rearrange("p r b -> p b r"),
                                start=(kt == 0), stop=(kt == KT - 1))
                    # exp straight off the score psum (first half overlaps
                    # the second half's matmuls on the PE)
                    for (r0, r1, sps) in halves:
                        nc.scalar.activation(ex16[:, :, r0:r1], sps[:], AF.Exp)

                    # fc pass-1 copy for the previous step's unit: split into
                    # four low-priority pieces so the scheduler slots them
                    # into idle DVE time without blocking the chain
                    if fc_pending is not None:
                        pfps, pot, pcols, pnv = fc_pending
                        with tc.high_priority(offset=-(10 ** 6)):
                            for qi in range(4):
                                a = qi * 256
                                b = min((qi + 1) * 256, pnv)
                                if a >= b:
                                    continue
                                if has_fcb:
                                    nc.vector.tensor_tensor(
                                        out=pot[:, a:b], in0=pfps[:, a:b],
                                        in1=fcb[:, pcols.start + a:
                                                pcols.start + b], op=OP.add)
                                else:
                                    nc.vector.tensor_copy(
                                        pot[:, a:b], pfps[:, a:b])
                            dma_eng = nc.sync if t % 2 == 0 else nc.scalar
                            dma_eng.dma_start(out_d.ap()[0:128, pcols],
                                              pot[:, :pnv])
                        fc_pending = None

                    # transpose exp(scores) to r-on-partitions: 4 K=1 matmuls
                    exflat = ex16[:].rearrange("p b r -> p (b r)")
                    for c in range(BL // 2):
                        nc.tensor.matmul(
                            scT[:, c:c + 1], exflat[:, c * 128:(c + 1) * 128],
                            one1[:], start=True, stop=True)
                    exTs = PST.tile([128, BL // 2], F16, name="exTs")
                    nc.vector.tensor_copy(exTs[:], scT[:])

                    # replicated softmax denominator (before ctx so the
                    # reciprocal overlaps the ctx matmuls), then context
                    nc.tensor.matmul(srep[:, 0:4], ones128[0:49, :],
                                     exTs[0:49, :], start=True, stop=True)
                    nc.tensor.matmul(srep[:, 4:8], ones128[64:113, :],
                                     exTs[64:113, :], start=True, stop=True)
                    for c in range(BL // 2):
                        for k in range(KT):
                            nc.tensor.matmul(
                                ctx_ps[:, k, 2 * c:2 * c + 1],
                                featsR[0:49, c, k * 128:(k + 1) * 128],
                                exTs[0:49, c:c + 1], start=True, stop=True)
                            nc.tensor.matmul(
                                ctx_ps[:, k, 2 * c + 1:2 * c + 2],
                                featsR[64:113, c, k * 128:(k + 1) * 128],
                                exTs[64:113, c:c + 1], start=True, stop=True)
                    # srep cols: [s0 s2 s4 s6 | s1 s3 s5 s7] (slot parity)
                    rec = PST.tile([128, BL], F32, name="rec")
                    nc.vector.reciprocal(rec[:], srep[:])
                    ctx16 = PST.tile([128, KT, BL], F16, name="ctx16")
                    nc.vector.tensor_tensor(
                        out=ctx16[:].rearrange("p k (j par) -> p k j par",
                                               par=2),
                        in0=ctx_ps[:].rearrange("p k (j par) -> p k j par",
                                                par=2),
                        in1=rec[:].rearrange("p (par j) -> p j par", par=2)[
                            :, None, :, :].to_broadcast((128, KT, 4, 2)),
                        op=OP.mult)

                    # cgx accumulates onto ghx; the very last matmul closes
                    # the bank's accumulation group
                    for m in range(M3H):
                        for kt in range(KT):
                            nc.tensor.matmul(
                                ghx[:, m], W_ihcT[:, kt, m * 128:(m + 1) * 128],
                                ctx16[:, kt], start=False,
                                stop=(m == M3H - 1 and kt == KT - 1))

                    # fc interleave: rows 0-127 during steps 16+; matmuls
                    # here (PE idles during the gate tail), copy next step
                    if t in fc1:
                        c0, c1 = fc1[t]
                        nv = min(512 * c1, V) - 512 * c0
                        cols = slice(c0 * 512, c0 * 512 + nv)
                        fps = PS_FC.tile([128, 1024], F32, name="fc_ps")
                        for ch in range(c0, c1):
                            cnv = min(512, V - ch * 512)
                            for kt in range(KT):
                                nc.tensor.matmul(
                                    fps[:, (ch - c0) * 512:(ch - c0) * 512 + cnv],
                                    h_all[:, kt, 0:128],
                                    fcW[:, kt, ch * 512:ch * 512 + cnv],
                                    start=(kt == 0), stop=(kt == KT - 1))
                        ot = FSB.tile([128, 1024], F32, name="fc_ot")
                        fc_pending = (fps, ot, cols, nv)

                    # gates. r,z = 0.5*(1+tanh(0.5*x)). ghx n-rows already
                    # hold xn + hn/2 + cgx_n, and hn_ps holds hn/2, so
                    # n_pre = ghx_n + tanh(0.5*x_r)*hn/2 with no affine fix.
                    rz_h = PST.tile([128, 8, BL], F32, name="rz_h")
                    nc.scalar.activation(rz_h[:], ghx[:, 0:8], AF.Tanh,
                                         scale=0.5)
                    npre = PST.tile([128, KT, BL], F32, name="npre")
                    t_r = PST.tile([128, KT, BL], F32, name="t_r")
                    nc.vector.tensor_tensor(
                        out=t_r[:], in0=rz_h[:, 0:4], in1=hn_ps[:], op=OP.mult)
                    nc.vector.tensor_tensor(
                        out=npre[:], in0=ghx[:, 8:12], in1=t_r[:], op=OP.add)
                    # z and 1-z, and z*h ahead of the final tanh
                    z_g = PST.tile([128, KT, BL], F16, name="z_g")
                    zc_g = PST.tile([128, KT, BL], F16, name="zc_g")
                    nc.vector.tensor_scalar(
                        out=z_g[:], in0=rz_h[:, 4:8], scalar1=0.5,
                        scalar2=0.5, op0=OP.mult, op1=OP.add)
                    nc.vector.tensor_scalar(
                        out=zc_g[:], in0=rz_h[:, 4:8], scalar1=-0.5,
                        scalar2=0.5, op0=OP.mult, op1=OP.add)
                    q_g = PST.tile([128, KT, BL], F16, name="q_g")
                    nc.vector.tensor_tensor(
                        out=q_g[:], in0=z_g[:], in1=h_prev, op=OP.mult)
                    n_t = PST.tile([128, KT, BL], F16, name="n_t")
                    nc.scalar.activation(n_t[:], npre[:], AF.Tanh)
                    # h_new = z*h + (1-z)*n, written straight into h_all
                    w_g = PST.tile([128, KT, BL], F16, name="w_g")
                    nc.vector.tensor_tensor(
                        out=w_g[:], in0=zc_g[:], in1=n_t[:], op=OP.mult)
                    nc.vector.tensor_tensor(
                        out=h_all[:, :, t * BL:(t + 1) * BL], in0=q_g[:],
                        in1=w_g[:], op=OP.add)

            # ---- fc pass 2: rows 128-255 ----
            with tc.tile_pool(name="fc2_ps", bufs=4, space="PSUM") as FPS2, \
                 tc.tile_pool(name="fc2_sb", bufs=4) as FSB2:
                for ch in range(NCH):
                    nv = min(512, V - ch * 512)
                    cols = slice(ch * 512, ch * 512 + nv)
                    ps = FPS2.tile([128, 512], F32, name="fc2_ps")
                    for kt in range(KT):
                        nc.tensor.matmul(
                            ps[:, :nv], h_all[:, kt, 128:256],
                            fcW[:, kt, cols], start=(kt == 0),
                            stop=(kt == KT - 1))
                    ot = FSB2.tile([128, 512], F32, name="fc2_ot")
                    if has_fcb:
                        nc.vector.tensor_tensor(
                            out=ot[:, :nv], in0=ps[:, :nv],
                            in1=fcb[:, cols], op=OP.add)
                    elif ch % 2 == 0:
                        nc.vector.tensor_copy(ot[:, :nv], ps[:, :nv])
                    else:
                        nc.scalar.copy(ot[:, :nv], ps[:, :nv])
                    dma_eng = nc.sync if ch % 2 == 0 else nc.scalar
                    dma_eng.dma_start(out_d.ap()[128:256, cols], ot[:, :nv])

    nc.compile()
    return nc


def _get_built(has_fcb=True):
    with _BUILD_LOCK:
        if has_fcb not in _BUILT:
            _BUILT[has_fcb] = _build(has_fcb)
    return _BUILT[has_fcb]


def kernel(features, captions, embed_table, attn_W, attn_b, v_w,
           W_ih, W_hh, b_ih, b_hh, fc_W, fc_b):
    from concourse.bass_utils import run_bass_kernel_spmd

    features = np.asarray(features, dtype=np.float32)
    captions = np.asarray(captions)
    embed_table = np.asarray(embed_table, dtype=np.float32)
    attn_W = np.asarray(attn_W, dtype=np.float32)
    attn_b = np.asarray(attn_b, dtype=np.float32)
    v_w = np.asarray(v_w, dtype=np.float32)
    W_ih = np.asarray(W_ih, dtype=np.float32)
    W_hh = np.asarray(W_hh, dtype=np.float32)
    b_ih = np.asarray(b_ih, dtype=np.float32)
    b_hh = np.asarray(b_hh, dtype=np.float32)
    fc_W = np.asarray(fc_W, dtype=np.float32)
    fc_b = np.asarray(fc_b, dtype=np.float32)

    has_fcb = bool(np.any(fc_b))
    nc = _get_built(has_fcb)

    f16 = np.float16
    shared = {
        "attn_We": _round_f32r(attn_W[:E]),
        "attn_Wh": attn_W[E:].astype(f16),
        "W_hhT": np.ascontiguousarray(W_hh[:2 * H].T).astype(f16),
        "W_hhn2": np.ascontiguousarray(0.5 * W_hh[2 * H:].T).astype(f16),
        "W_ihcT": np.ascontiguousarray(W_ih[:, E:].T).astype(f16),
        "W_iheT": _round_f32r(W_ih[:, :E].T),
        "vw": v_w[:, None].astype(f16),
        "bsum": (b_ih + b_hh)[None, :].astype(f16),
        "attnb": np.ascontiguousarray(attn_b[:, None]),
        "fcW": fc_W.astype(f16),
    }
    if has_fcb:
        shared["fcb"] = fc_b[None, :].astype(f16)
    emb = embed_table[captions[:, :T].astype(np.int64)]  # [B, T, E]
    perm = np.array(PERM)
    in_maps = []
    for c in range(NCORES):
        rows = c * BL + perm             # batch rows in slot order
        fr = features[rows]              # [BL, R, E] slot-ordered
        featsR = np.zeros((128, BL // 2, E), dtype=np.float32)
        for cc in range(BL // 2):
            featsR[0:49, cc] = fr[2 * cc]
            featsR[64:113, cc] = fr[2 * cc + 1]
        m = dict(shared)
        m["featsT"] = _round_f32r(fr.transpose(2, 1, 0))
        m["featsR"] = featsR.astype(f16)
        m["embT"] = _round_f32r(
            emb[rows].transpose(2, 1, 0).reshape(E, T * BL))
        in_maps.append(m)

    res = run_bass_kernel_spmd(nc, in_maps, core_ids=list(range(NCORES)))

    out = np.empty((B, T, V), dtype=np.float32)
    for c in range(NCORES):
        # per-core output rows are t*BL + slot; slot i is batch PERM[i]
        r = res.results[c]["out"].reshape(T, BL, V)
        out[c * BL + perm] = r.transpose(1, 0, 2)
    return out
